# revision 15
# baseline (speedup 1.0000x reference)
"""Distributed Trainium2 kernel for a dense-transformer attention block.

Math (matches the reference):
    xqkv = x @ Wqkv + bqkv ; split into q,k,v heads
    scores = (q k^T) / sqrt(HD) + mask ; attn = softmax(scores)
    o = attn @ v ; out = o @ Wproj + bproj

Parallelization over 8 NeuronCores:
  - QKV projection is DATA-parallel: each core projects its own 1/8 of
    the tokens against the FULL Wqkv.  An AllToAll then regroups q/k/v
    so each core holds 2 heads over ALL tokens (head-parallel
    attention).  A second AllToAll redistributes the per-head outputs
    so each core holds ALL head-dims for its 1/8 of the rows and runs
    the output projection; the host concatenates row blocks.

Performance structure (v2):
  - x is pre-cast + pre-transposed on the HOST, so the kernel's first
    matmul only waits for a single 4MiB DMA (~15us) instead of an
    on-chip cast/round-trip-transpose (~57us).
  - Phase-1 order is v -> A2A(v) -> q -> A2A(q) -> kA -> A2A(kA) -> kB
    -> A2A(kB): every input the first attention unit needs has finished
    its collective before the QKV matmuls end, so TensorE never idles
    between phases.  v's stationaries are xT slices reused 4x.
  - Attention runs as ONE continuous 2-deep software pipeline across
    all (b, head, chunk) sections; collectives are issued mid-stream
    and tracked by Tile semaphores.
  - Wproj is preloaded during attention; the first output-projection
    rows are interleaved into the attention tail so the final A2A's
    latency hides behind real matmuls.  Projection PSUM comes from the
    same pool ring as the attention score tiles (two 512-wide chunks
    per [128,1024] tile; matmuls never cross a PSUM bank).
  - Softmax row-sums are ones-stationary matmuls (broadcast across
    partitions); normalization uses the fast DVE reciprocal (~18-bit,
    plenty for the 2e-2 gate).
"""

import hashlib
import numpy as np
import ml_dtypes

B, S, DIM, NH = 4, 2048, 2048, 16
HD = DIM // NH  # 128
NCORES = 8
HPC = NH // NCORES          # heads per core = 2
TOK = B * S                 # 8192 tokens
RPC = TOK // NCORES         # rows (tokens) per core = 1024
CH = 512                    # attention chunk (q and kv)
SUB = 128                   # kv subtile
SCALE = 1.0 / float(np.sqrt(HD))

_BF16 = ml_dtypes.bfloat16

_prog_cache = {}


def _analyze_mask(mask):
    """Build the attention schedule from the additive mask.

    sched[qc] = list of (kc, j, q_lo, mask_id, c_lo, c_hi); mask_id is
    -1 when no mask add is needed for the entry.  Mask tiles are already
    transposed to [kv, q] layout and pre-divided by SCALE.
    """
    m = np.asarray(mask, dtype=np.float32).reshape(S, S)
    NEG = -1e8
    sched = []
    tiles = []
    tile_key = {}
    for qc in range(S // CH):
        ents = []
        for kc in range(S // CH):
            blk = m[qc * CH:(qc + 1) * CH, kc * CH:(kc + 1) * CH]
            if np.all(blk <= NEG):
                continue
            for j in range(CH // SUB):
                sub = blk[:, j * SUB:(j + 1) * SUB]       # [CH q, SUB kv]
                if np.all(sub <= NEG):
                    continue
                vis = ~np.all(sub <= NEG, axis=1)
                q_lo = int(np.argmax(vis))
                q_lo = (q_lo // SUB) * SUB
                if not ents:
                    q_lo = 0  # first entry must initialize full PSUM width
                nzrow = np.any(sub[q_lo:, :] != 0.0, axis=1)
                if nzrow.any():
                    first = q_lo + int(np.argmax(nzrow))
                    last = q_lo + len(nzrow) - int(np.argmax(nzrow[::-1]))
                    c_lo = (first // SUB) * SUB
                    c_hi = min(CH, ((last + SUB - 1) // SUB) * SUB)
                    content = np.ascontiguousarray(
                        (sub[c_lo:c_hi, :].T / SCALE).astype(_BF16))
                    key = (c_hi - c_lo,
                           hashlib.md5(content.tobytes()).hexdigest())
                    if key not in tile_key:
                        tile_key[key] = len(tiles)
                        tiles.append(content)
                    ents.append((kc, j, q_lo, tile_key[key], c_lo, c_hi))
                else:
                    ents.append((kc, j, q_lo, -1, 0, 0))
        assert ents, "a full query chunk is masked out; softmax undefined"
        sched.append(ents)
    n_real = len(tiles)
    widths = [t.shape[1] for t in tiles]
    pack = np.zeros((max(1, n_real), SUB, CH), dtype=_BF16)
    for i, t in enumerate(tiles):
        pack[i, :, :t.shape[1]] = t
    return sched, pack, widths, n_real


def _build_program(sched, n_mask_tiles, mask_widths):
    import concourse.bass as bass
    import concourse.tile as tile
    from concourse import bacc, mybir
    from contextlib import ExitStack

    f32 = mybir.dt.float32
    bf16 = mybir.dt.bfloat16
    AF = mybir.ActivationFunctionType
    ALU = mybir.AluOpType

    nc = bacc.Bacc("TRN2", target_bir_lowering=False, debug=False,
                   num_devices=NCORES)

    xt_ext = nc.dram_tensor("xt", [128, DIM // 128, RPC], bf16,
                            kind="ExternalInput").ap()
    wqk_ext = nc.dram_tensor("wqk", [2 * DIM // 128, 128, DIM // 128, 128],
                             bf16, kind="ExternalInput").ap()
    wv_ext = nc.dram_tensor("wv", [DIM // CH, DIM // 128, 128, CH], bf16,
                            kind="ExternalInput").ap()
    bqk_ext = nc.dram_tensor("bqk", [128, 2 * DIM // 128], f32,
                             kind="ExternalInput").ap()
    bv_ext = nc.dram_tensor("bv", [1, DIM], f32, kind="ExternalInput").ap()
    maskt_ext = nc.dram_tensor("maskt", [max(1, n_mask_tiles), SUB, CH], bf16,
                               kind="ExternalInput").ap()
    wproj_ext = nc.dram_tensor("wproj", [DIM // 128, DIM // CH, 128, CH],
                               bf16, kind="ExternalInput").ap()
    bproj_ext = nc.dram_tensor("bproj", [1, DIM], f32,
                               kind="ExternalInput").ap()
    out_ext = nc.dram_tensor("out", [RPC, DIM], bf16,
                             kind="ExternalOutput").ap()

    NDT = DIM // 128          # 16 contraction tiles
    NSC = S // CH             # 4 s-chunks per batch
    NQKM = 2 * DIM // 128     # 32 q/k output M-tiles (dest-grouped)
    rg = [list(range(NCORES))]

    with tile.TileContext(nc) as tc, ExitStack() as top:
        dram = top.enter_context(tc.tile_pool(name="dram", bufs=1,
                                              space="DRAM"))
        q_send = dram.tile([NCORES, HPC, 2, 128, CH], bf16,
                           name="q_send")
        k_sendA = dram.tile([NCORES, 2, 128, CH], bf16, name="k_sendA")
        k_sendB = dram.tile([NCORES, 2, 128, CH], bf16, name="k_sendB")
        v_send = dram.tile([NCORES, 128, HPC, RPC // 128, HD], bf16,
                           name="v_send")
        q_out = dram.tile([NCORES, HPC, 2, 128, CH], bf16,
                          name="q_out")
        k_outA = dram.tile([NCORES, 2, 128, CH], bf16, name="k_outA")
        k_outB = dram.tile([NCORES, 2, 128, CH], bf16, name="k_outB")
        v_out = dram.tile([NCORES, 128, HPC, RPC // 128, HD], bf16,
                          name="v_out")
        a2a_in = [dram.tile([DIM, RPC // 2], bf16, name=f"a2a_in{i}")
                  for i in range(2)]
        a2a_out = [dram.tile([DIM, RPC // 2], bf16, name=f"a2a_out{i}")
                   for i in range(2)]

        const = top.enter_context(tc.tile_pool(name="const", bufs=1))
        ones = const.tile([128, 128], bf16, name="ones", tag="ones")
        nc.any.memset(ones[:], 1.0)
        msk = []
        for i in range(n_mask_tiles):
            w = mask_widths[i]
            mt = const.tile([128, w], bf16, name=f"msk{i}", tag=f"msk{i}")
            msk.append(mt)
        # broadcast biases (filled early in phase 1; bf16 is plenty
        # for a +bias epilogue)
        bvb = const.tile([128, DIM], bf16, name="bvb", tag="bvb")
        bpb = const.tile([128, DIM], bf16, name="bpb", tag="bpb")

        # persistent qkv storage for the attention phase (bf16); loads are
        # issued during phase 1 as soon as each AllToAll lands.
        qT = [[None] * HPC for _ in range(B)]
        kT = [[None] * HPC for _ in range(B)]
        vS = [[None] * HPC for _ in range(B)]
        frees = []
        for b in range(B):
            for h in range(HPC):
                t1, f1 = tc.tile([128, S], bf16, name=f"qT{b}{h}")
                t2, f2 = tc.tile([128, S], bf16, name=f"kT{b}{h}")
                t3, f3 = tc.tile([128, S], bf16, name=f"vS{b}{h}")
                qT[b][h], kT[b][h], vS[b][h] = t1, t2, t3
                frees += [f1, f2, f3]

        # ================= Phase 1: data-parallel QKV projection ======
        with ExitStack() as p1:
            bqkp = p1.enter_context(tc.tile_pool(name="bqkp", bufs=1))
            bmall = bqkp.tile([128, NQKM], f32, name="bmall", tag="bmall")
            with ExitStack() as bs:
                biasp = bs.enter_context(tc.tile_pool(name="biasp",
                                                      bufs=1))
                bv1 = biasp.tile([1, DIM], f32, name="bv1", tag="bv1")
                nc.sync.dma_start(out=bv1[:], in_=bv_ext[:, :])
                bv1h = biasp.tile([1, DIM], bf16, name="bv1h", tag="bv1h")
                nc.vector.tensor_copy(bv1h[:], bv1[:])
                nc.gpsimd.partition_broadcast(bvb[:], bv1h[:])
                bp1 = biasp.tile([1, DIM], f32, name="bp1", tag="bp1")
                nc.sync.dma_start(out=bp1[:], in_=bproj_ext[:, :])
                bp1h = biasp.tile([1, DIM], bf16, name="bp1h", tag="bp1h")
                nc.vector.tensor_copy(bp1h[:], bp1[:])
                nc.gpsimd.partition_broadcast(bpb[:], bp1h[:])
            xtp = p1.enter_context(tc.tile_pool(name="xtp", bufs=1))
            xT = xtp.tile([128, NDT, RPC], bf16, name="xT", tag="xT")
            nc.sync.dma_start(out=xT[:, :, 0:RPC // 2],
                              in_=xt_ext[:, :, 0:RPC // 2])
            nc.gpsimd.dma_start(out=xT[:, :, RPC // 2:RPC],
                               in_=xt_ext[:, :, RPC // 2:RPC])

            psA = p1.enter_context(tc.tile_pool(name="psA", bufs=3,
                                                space="PSUM"))
            psV = p1.enter_context(tc.tile_pool(name="psV", bufs=3,
                                                space="PSUM"))
            bqkp = p1.enter_context(tc.tile_pool(name="bqkp", bufs=4))
            wqkp = p1.enter_context(tc.tile_pool(name="wqkp", bufs=9))
            qksb = p1.enter_context(tc.tile_pool(name="qksb", bufs=4))
            wvp = p1.enter_context(tc.tile_pool(name="wvp", bufs=1))
            vsb = p1.enter_context(tc.tile_pool(name="vsb", bufs=4))

            # --- v first: its AllToAll must land before attention starts
            bm_loaded = False
            for dhc in range(DIM // CH):
                wvs = []
                for dt_i in range(NDT):
                    wv = wvp.tile([128, CH], bf16, name=f"wv{dt_i}",
                                  tag=f"wv{dt_i}")
                    nc.sync.dma_start(out=wv[:], in_=wv_ext[dhc, dt_i])
                    wvs.append(wv)
                if not bm_loaded:
                    bm_loaded = True
                    nc.sync.dma_start(out=bmall[:], in_=bqk_ext[:, :])
                    for i in range(n_mask_tiles):
                        nc.sync.dma_start(
                            out=msk[i][:],
                            in_=maskt_ext[i, :, :mask_widths[i]])
                for tt in range(RPC // 128):
                    ps = psV.tile([128, CH], f32, name="psv", tag="V")
                    for dt_i in range(NDT):
                        nc.tensor.matmul(
                            ps[:],
                            xT[:, dt_i, tt * 128:(tt + 1) * 128],
                            wvs[dt_i][:],
                            start=(dt_i == 0), stop=(dt_i == NDT - 1))
                    sb = vsb.tile([128, CH], bf16, name="vsbt", tag="vsbt")
                    nc.vector.scalar_tensor_tensor(
                        out=sb[:], in0=ps[:], scalar=1.0,
                        in1=bvb[:, dhc * CH:(dhc + 1) * CH],
                        op0=ALU.mult, op1=ALU.add)
                    # split the 512 v-dims into the two destination
                    # blocks, stored so the attention-side read of each
                    # head is a fully contiguous [128, 1024] block
                    for half in range(2):
                        dest = 2 * dhc + half
                        for hp in range(HPC):
                            nc.scalar.dma_start(
                                out=v_send[dest, :, hp, tt, :],
                                in_=sb[:, (2 * half + hp) * HD:
                                       (2 * half + hp + 1) * HD])
            nc.gpsimd.collective_compute(
                "AllToAll", mybir.AluOpType.bypass, replica_groups=rg,
                ins=[v_send.opt()], outs=[v_out.opt()])

            def qk_tile(m):
                # one dest-grouped M-tile of [dim, 128]; m<16: q,
                # 16..23: k head-0 half, 24..31: k head-1 half
                wm = wqkp.tile([128, DIM], bf16, name="wm", tag="wm")
                nc.sync.dma_start(
                    out=wm[:].rearrange("p (dt c) -> p dt c", dt=NDT),
                    in_=wqk_ext[m])
                for nchk in range(RPC // CH):
                    ps = psA.tile([128, CH], f32, name="psqk", tag="A")
                    for dt_i in range(NDT):
                        nc.tensor.matmul(
                            ps[:], wm[:, dt_i * 128:(dt_i + 1) * 128],
                            xT[:, dt_i, nchk * CH:(nchk + 1) * CH],
                            start=(dt_i == 0), stop=(dt_i == NDT - 1))
                    sb = qksb.tile([128, CH], bf16, name="sb", tag="sb")
                    nc.vector.tensor_scalar_add(sb[:], ps[:], bmall[:, m:m + 1])
                    if m < NQKM // 2:
                        dst = q_send[m // HPC, m % HPC, nchk, :, :]
                    elif m < 3 * NQKM // 4:
                        dst = k_sendA[m - NQKM // 2, nchk, :, :]
                    else:
                        dst = k_sendB[m - 3 * NQKM // 4, nchk, :, :]
                    nc.scalar.dma_start(out=dst, in_=sb[:])

            for m in range(NQKM // 2):
                qk_tile(m)
            nc.gpsimd.collective_compute(
                "AllToAll", mybir.AluOpType.bypass, replica_groups=rg,
                ins=[q_send.opt()], outs=[q_out.opt()])
            for b in range(B):
                for sh in range(2):
                    nc.gpsimd.dma_start(
                        out=vS[b][0][:, sh * RPC:(sh + 1) * RPC],
                        in_=v_out[2 * b + sh, :, 0, :, :])
            for m in range(NQKM // 2, 3 * NQKM // 4):
                qk_tile(m)
            nc.gpsimd.collective_compute(
                "AllToAll", mybir.AluOpType.bypass, replica_groups=rg,
                ins=[k_sendA.opt()], outs=[k_outA.opt()])
            for b in range(B):
                for sh in range(2):
                    for c in range(2):
                        nc.gpsimd.dma_start(
                            out=qT[b][0][:, sh * RPC + c * CH:
                                         sh * RPC + (c + 1) * CH],
                            in_=q_out[2 * b + sh, 0, c])
            for b in range(B):
                for sh in range(2):
                    for c in range(2):
                        nc.gpsimd.dma_start(
                            out=kT[b][0][:, sh * RPC + c * CH:
                                         sh * RPC + (c + 1) * CH],
                            in_=k_outA[2 * b + sh, c])
            for m in range(3 * NQKM // 4, NQKM):
                qk_tile(m)
            nc.gpsimd.collective_compute(
                "AllToAll", mybir.AluOpType.bypass, replica_groups=rg,
                ins=[k_sendB.opt()], outs=[k_outB.opt()])

            for b in range(B):
                for sh in range(2):
                    nc.gpsimd.dma_start(
                        out=vS[b][1][:, sh * RPC:(sh + 1) * RPC],
                        in_=v_out[2 * b + sh, :, 1, :, :])
                    for c in range(2):
                        nc.gpsimd.dma_start(
                            out=qT[b][1][:, sh * RPC + c * CH:
                                         sh * RPC + (c + 1) * CH],
                            in_=q_out[2 * b + sh, 1, c])
            for b in range(B):
                for sh in range(2):
                    for c in range(2):
                        nc.gpsimd.dma_start(
                            out=kT[b][1][:, sh * RPC + c * CH:
                                         sh * RPC + (c + 1) * CH],
                            in_=k_outB[2 * b + sh, c])

        # ================= Phase 2: attention + interleaved proj ======
        with ExitStack() as p2:
            psG = p2.enter_context(tc.tile_pool(name="psG", bufs=2,
                                                space="PSUM"))
            psO = p2.enter_context(tc.tile_pool(name="psO", bufs=2,
                                                space="PSUM"))
            psS = p2.enter_context(tc.tile_pool(name="psS", bufs=2,
                                                space="PSUM"))
            # output projection weights, preloaded during attention
            wpp = p2.enter_context(tc.tile_pool(name="wpp", bufs=1))
            wps = []  # [ot][nchunk] -> [128, CH]
            for ot in range(NDT):
                row = []
                for nchunk in range(DIM // CH):
                    wp = wpp.tile([128, CH], bf16, name=f"wp{ot}_{nchunk}",
                                  tag=f"wp{ot}_{nchunk}")
                    nc.sync.dma_start(out=wp[:],
                                      in_=wproj_ext[ot, nchunk])
                    row.append(wp)
                wps.append(row)
            p2a = p2.enter_context(ExitStack())
            ptp = p2a.enter_context(tc.tile_pool(name="ptp", bufs=5))
            recp = p2a.enter_context(tc.tile_pool(name="recp", bufs=6))
            otp = p2a.enter_context(tc.tile_pool(name="otp", bufs=8))

            state = {}  # (b,h,qc) -> (o_ps, s_sum)

            def front(b, h, qc, p0):
                ents = sched[qc]
                if p0 == 0:
                    state[(b, h, qc)] = (
                        psO.tile([128, CH], f32, name="o_ps", tag="O"),
                        psS.tile([128, CH], f32, name="s_sum", tag="Ssum"))
                n = min(2, len(ents) - p0)
                spsG = psG.tile([128, 2 * CH], f32, name="spsG", tag="G")
                pT = ptp.tile([128, 2 * CH], bf16, name="pT", tag="pT")
                g_lo = None
                for i in range(n):
                    (kc, j, q_lo, mid, c_lo, c_hi) = ents[p0 + i]
                    kv0 = kc * CH + j * SUB
                    off = i * CH
                    nc.tensor.matmul(
                        spsG[:, off + q_lo:off + CH],
                        kT[b][h][:, kv0:kv0 + SUB],
                        qT[b][h][:, qc * CH + q_lo:(qc + 1) * CH],
                        start=True, stop=True)
                    if mid >= 0:
                        nc.vector.tensor_add(
                            spsG[:, off + c_lo:off + c_hi],
                            spsG[:, off + c_lo:off + c_hi],
                            msk[mid][:, :c_hi - c_lo])
                    lo = off + q_lo
                    g_lo = lo if g_lo is None else min(g_lo, lo)
                nc.scalar.activation(
                    pT[:, g_lo:n * CH], spsG[:, g_lo:n * CH],
                    AF.Exp, scale=SCALE)
                return pT

            def back(b, h, qc, p0, pT):
                ents = sched[qc]
                last = len(ents) - 1
                o_ps, s_sum = state[(b, h, qc)]
                n = min(2, len(ents) - p0)
                for i in range(n):
                    ei = p0 + i
                    (kc, j, q_lo, mid, c_lo, c_hi) = ents[ei]
                    off = i * CH
                    nc.tensor.matmul(
                        s_sum[:, q_lo:CH], ones[:],
                        pT[:, off + q_lo:off + CH],
                        start=(ei == 0), stop=(ei == last))
                    kvt = kc * (CH // SUB) + j
                    nc.tensor.matmul(
                        o_ps[:, q_lo:CH],
                        vS[b][h][:, kvt * HD:(kvt + 1) * HD],
                        pT[:, off + q_lo:off + CH],
                        start=(ei == 0), stop=(ei == last))
                if p0 + n > last:  # query chunk complete
                    del state[(b, h, qc)]
                    rec = recp.tile([128, CH], f32, name="rec", tag="rec")
                    nc.vector.reciprocal_approx_fast(rec[:], s_sum[:])
                    oT = otp.tile([128, CH], bf16, name="oT", tag="oT")
                    nc.vector.tensor_mul(oT[:], o_ps[:], rec[:])
                    dest = 2 * b + qc // 2
                    r0 = dest * (HPC * HD) + h * HD
                    nc.scalar.dma_start(
                        out=a2a_in[qc % 2][r0:r0 + HD, :], in_=oT[:])

            pipe = []

            def run_units(items):
                # extend the single software pipeline (never drains
                # between sections)
                for (b, h, qcs) in items:
                    for qc in qcs:
                        for p0 in range(0, len(sched[qc]), 2):
                            u = (b, h, qc, p0)
                            pipe.append((u, front(*u)))
                            if len(pipe) > 1:
                                (ub, pT) = pipe.pop(0)
                                back(*ub, pT)

            def drain():
                while pipe:
                    (ub, pT) = pipe.pop(0)
                    back(*ub, pT)

            oc_t = [None] * NDT
            ocp = None
            resp = None

            def proj_load(half):
                for ot in range(NDT):
                    t = ocp.tile([128, RPC // 2], bf16,
                                 name=f"oc{ot}", tag=f"oc{ot}")
                    nc.sync.dma_start(
                        out=t[:],
                        in_=a2a_out[half][ot * 128:(ot + 1) * 128, :])
                    oc_t[ot] = t

            def proj_rts(half, rts):
                # output projection for row-tiles rts of the given half;
                # PSUM comes from the psG ring (2x 512-wide chunks per
                # [128,1024] tile, bank-aligned).
                for rt in rts:
                    for pair in range(2):
                        t = psG.tile([128, 2 * CH], f32, name="spsG",
                                     tag="G")
                        for sub_i in range(2):
                            nchunk = pair * 2 + sub_i
                            for ot in range(NDT):
                                nc.tensor.matmul(
                                    t[:, sub_i * CH:(sub_i + 1) * CH],
                                    oc_t[ot][:, rt * 128:(rt + 1) * 128],
                                    wps[ot][nchunk][:],
                                    start=(ot == 0), stop=(ot == NDT - 1))
                        grow = half * (RPC // 2) + rt * 128
                        for sub_i in range(2):
                            nchunk = pair * 2 + sub_i
                            res = resp.tile([128, CH], bf16, name="res",
                                            tag="res")
                            nc.vector.scalar_tensor_tensor(
                                out=res[:],
                                in0=t[:, sub_i * CH:(sub_i + 1) * CH],
                                scalar=1.0,
                                in1=bpb[:, nchunk * CH:(nchunk + 1) * CH],
                                op0=ALU.mult, op1=ALU.add)
                            nc.sync.dma_start(
                                out=out_ext[grow:grow + 128,
                                            nchunk * CH:(nchunk + 1) * CH],
                                in_=res[:])

            # S1: head 0, even query chunks; S2: head 1, even chunks
            run_units([(b, 0, (0, 2)) for b in range(B)])
            run_units([(b, 1, (0, 2)) for b in range(B)])
            drain()
            nc.gpsimd.collective_compute(
                "AllToAll", mybir.AluOpType.bypass, replica_groups=rg,
                ins=[a2a_in[0].opt()], outs=[a2a_out[0].opt()])
            # S3: head 0, odd chunks; then most of proj half-0 (its A2A
            # lands while S3 computes); S4: head 1, odd chunks.
            run_units([(b, 0, (1, 3)) for b in range(B)])
            run_units([(b, 1, (1, 3)) for b in range(B)])
            drain()
            nc.gpsimd.collective_compute(
                "AllToAll", mybir.AluOpType.bypass, replica_groups=rg,
                ins=[a2a_in[1].opt()], outs=[a2a_out[1].opt()])
            # attention staging pools are dead now; free them so the
            # projection staging fits
            p2a.close()
            ocp = p2.enter_context(tc.tile_pool(name="ocp", bufs=1))
            resp = p2.enter_context(tc.tile_pool(name="resp", bufs=4))
            # proj half-0 (inputs long since landed) covers the final
            # A2A's transfer; half-1 follows with everything resident.
            proj_load(0)
            proj_rts(0, [0, 1, 2, 3])
            proj_load(1)
            proj_rts(1, [0, 1, 2, 3])

        for f in reversed(frees):
            f()

    nc.compile()
    return nc


def _get_program(sched, n_real, mask_widths):
    key = (str(sched), tuple(mask_widths))
    if key not in _prog_cache:
        _prog_cache[key] = _build_program(sched, n_real, mask_widths)
    return _prog_cache[key]


def kernel(x=None, mask=None, Wqkv=None, bqkv=None, Wproj=None, bproj=None,
           start_pos=0, **_unused):
    from concourse.bass_utils import run_bass_kernel_spmd

    x = np.ascontiguousarray(np.asarray(x, dtype=np.float32).reshape(TOK, DIM))
    mask = np.asarray(mask, dtype=np.float32)
    Wqkv = np.asarray(Wqkv, dtype=np.float32)
    bqkv = np.asarray(bqkv, dtype=np.float32)
    Wproj = np.asarray(Wproj, dtype=np.float32)
    bproj = np.asarray(bproj, dtype=np.float32)

    sched, mask_pack, widths, n_real = _analyze_mask(mask)
    nc = _get_program(sched, n_real, widths)

    # q/k weight columns reordered dest-major: for each destination core
    # d: [q_{2d}, q_{2d+1}, k_{2d}, k_{2d+1}]
    qk_cols = []
    for d in range(NCORES):
        for hh in (HPC * d, HPC * d + 1):
            qk_cols.append((hh * HD, (hh + 1) * HD))          # q
    for hl in range(HPC):
        for d in range(NCORES):
            hh = HPC * d + hl
            qk_cols.append((DIM + hh * HD, DIM + (hh + 1) * HD))  # k
    wqk = np.concatenate([Wqkv[:, a:b] for a, b in qk_cols], axis=1)
    bqk = np.concatenate([bqkv[a:b] for a, b in qk_cols])
    wv = Wqkv[:, 2 * DIM:]
    bv = bqkv[2 * DIM:]

    # pre-tile every weight on the host so each on-chip DMA is one
    # contiguous block (descriptor floods starve the DMA engines)
    wqk_t = wqk.astype(_BF16).reshape(DIM // 128, 128, 2 * DIM // 128, 128)
    wqk_t = wqk_t.transpose(2, 1, 0, 3)            # [m, p, dt, c]
    wv_t = wv.astype(_BF16).reshape(DIM // 128, 128, DIM // CH, CH)
    wv_t = wv_t.transpose(2, 0, 1, 3)              # [dhc, dt, p, c]
    wp_t = Wproj.astype(_BF16).reshape(DIM // 128, 128, DIM // CH, CH)
    wp_t = wp_t.transpose(0, 2, 1, 3)              # [ot, nc, p, c]
    shared = {
        "wqk": np.ascontiguousarray(wqk_t),
        "wv": np.ascontiguousarray(wv_t),
        "bqk": np.ascontiguousarray(
            bqk.astype(np.float32).reshape(-1, 128).T),
        "bv": np.ascontiguousarray(bv.reshape(1, DIM)),
        "maskt": mask_pack,
        "wproj": np.ascontiguousarray(wp_t),
        "bproj": np.ascontiguousarray(bproj.reshape(1, DIM)),
    }
    in_maps = []
    for c in range(NCORES):
        m = dict(shared)
        xc = x[c * RPC:(c + 1) * RPC].T.astype(_BF16)
        m["xt"] = np.ascontiguousarray(
            xc.reshape(DIM // 128, 128, RPC).transpose(1, 0, 2))
        in_maps.append(m)

    import os
    kw = {}
    if os.environ.get("KERNEL_TRACE"):
        kw["trace"] = True
    res = run_bass_kernel_spmd(nc, in_maps, core_ids=list(range(NCORES)), **kw)
    globals()["LAST_RUN"] = res
    if getattr(res, "exec_time_ns", None):
        print(f"HW exec time: {res.exec_time_ns} ns")
    outs = [res.results[c]["out"].astype(np.float32) for c in range(NCORES)]
    full = np.concatenate(outs, axis=0).reshape(B, S, DIM)
    return full


# revision 19
# speedup vs baseline: 1.0652x; 1.0652x over previous
"""Distributed Trainium2 kernel for a dense-transformer attention block.

Math (matches the reference):
    xqkv = x @ Wqkv + bqkv ; split into q,k,v heads
    scores = (q k^T) / sqrt(HD) + mask ; attn = softmax(scores)
    o = attn @ v ; out = o @ Wproj + bproj

Parallelization over 8 NeuronCores:
  - QKV projection is DATA-parallel: each core projects its own 1/8 of
    the tokens against the FULL Wqkv.  An AllToAll then regroups q/k/v
    so each core holds 2 heads over ALL tokens (head-parallel
    attention).  A second AllToAll redistributes the per-head outputs
    so each core holds ALL head-dims for its 1/8 of the rows and runs
    the output projection; the host concatenates row blocks.

Performance structure (v2):
  - x is pre-cast + pre-transposed on the HOST, so the kernel's first
    matmul only waits for a single 4MiB DMA (~15us) instead of an
    on-chip cast/round-trip-transpose (~57us).
  - Phase-1 order is v -> A2A(v) -> q -> A2A(q) -> kA -> A2A(kA) -> kB
    -> A2A(kB): every input the first attention unit needs has finished
    its collective before the QKV matmuls end, so TensorE never idles
    between phases.  v's stationaries are xT slices reused 4x.
  - Attention runs as ONE continuous 2-deep software pipeline across
    all (b, head, chunk) sections; collectives are issued mid-stream
    and tracked by Tile semaphores.
  - Wproj is preloaded during attention; the first output-projection
    rows are interleaved into the attention tail so the final A2A's
    latency hides behind real matmuls.  Projection PSUM comes from the
    same pool ring as the attention score tiles (two 512-wide chunks
    per [128,1024] tile; matmuls never cross a PSUM bank).
  - Softmax row-sums are ones-stationary matmuls (broadcast across
    partitions); normalization uses the fast DVE reciprocal (~18-bit,
    plenty for the 2e-2 gate).
"""

import hashlib
import numpy as np
import ml_dtypes

B, S, DIM, NH = 4, 2048, 2048, 16
HD = DIM // NH  # 128
NCORES = 8
HPC = NH // NCORES          # heads per core = 2
TOK = B * S                 # 8192 tokens
RPC = TOK // NCORES         # rows (tokens) per core = 1024
CH = 512                    # attention chunk (q and kv)
SUB = 128                   # kv subtile
SCALE = 1.0 / float(np.sqrt(HD))

_BF16 = ml_dtypes.bfloat16

_prog_cache = {}


def _analyze_mask(mask):
    """Build the attention schedule from the additive mask.

    sched[qc] = list of (kc, j, q_lo, mask_id, c_lo, c_hi); mask_id is
    -1 when no mask add is needed for the entry.  Mask tiles are already
    transposed to [kv, q] layout and pre-divided by SCALE.
    """
    m = np.asarray(mask, dtype=np.float32).reshape(S, S)
    NEG = -1e8
    sched = []
    tiles = []
    tile_key = {}
    for qc in range(S // CH):
        ents = []
        for kc in range(S // CH):
            blk = m[qc * CH:(qc + 1) * CH, kc * CH:(kc + 1) * CH]
            if np.all(blk <= NEG):
                continue
            for j in range(CH // SUB):
                sub = blk[:, j * SUB:(j + 1) * SUB]       # [CH q, SUB kv]
                if np.all(sub <= NEG):
                    continue
                vis = ~np.all(sub <= NEG, axis=1)
                q_lo = int(np.argmax(vis))
                q_lo = (q_lo // SUB) * SUB
                if not ents:
                    q_lo = 0  # first entry must initialize full PSUM width
                nzrow = np.any(sub[q_lo:, :] != 0.0, axis=1)
                if nzrow.any():
                    first = q_lo + int(np.argmax(nzrow))
                    last = q_lo + len(nzrow) - int(np.argmax(nzrow[::-1]))
                    c_lo = (first // SUB) * SUB
                    c_hi = min(CH, ((last + SUB - 1) // SUB) * SUB)
                    content = np.ascontiguousarray(
                        (sub[c_lo:c_hi, :].T / SCALE).astype(_BF16))
                    key = (c_hi - c_lo,
                           hashlib.md5(content.tobytes()).hexdigest())
                    if key not in tile_key:
                        tile_key[key] = len(tiles)
                        tiles.append(content)
                    ents.append((kc, j, q_lo, tile_key[key], c_lo, c_hi))
                else:
                    ents.append((kc, j, q_lo, -1, 0, 0))
        assert ents, "a full query chunk is masked out; softmax undefined"
        sched.append(ents)
    n_real = len(tiles)
    widths = [t.shape[1] for t in tiles]
    pack = np.zeros((max(1, n_real), SUB, CH), dtype=_BF16)
    for i, t in enumerate(tiles):
        pack[i, :, :t.shape[1]] = t
    return sched, pack, widths, n_real


def _build_program(sched, n_mask_tiles, mask_widths):
    import concourse.bass as bass
    import concourse.tile as tile
    from concourse import bacc, mybir
    from contextlib import ExitStack

    f32 = mybir.dt.float32
    bf16 = mybir.dt.bfloat16
    AF = mybir.ActivationFunctionType
    ALU = mybir.AluOpType

    nc = bacc.Bacc("TRN2", target_bir_lowering=False, debug=False,
                   num_devices=NCORES)

    xt_ext = nc.dram_tensor("xt", [128, DIM // 128, RPC], bf16,
                            kind="ExternalInput").ap()
    wqk_ext = nc.dram_tensor("wqk", [2 * DIM // 128, 128, DIM // 128, 128],
                             bf16, kind="ExternalInput").ap()
    wv_ext = nc.dram_tensor("wv", [DIM // CH, DIM // 128, 128, CH], bf16,
                            kind="ExternalInput").ap()
    bqk_ext = nc.dram_tensor("bqk", [128, 2 * DIM // 128], f32,
                             kind="ExternalInput").ap()
    bv_ext = nc.dram_tensor("bv", [1, DIM], f32, kind="ExternalInput").ap()
    maskt_ext = nc.dram_tensor("maskt", [max(1, n_mask_tiles), SUB, CH], bf16,
                               kind="ExternalInput").ap()
    wproj_ext = nc.dram_tensor("wproj", [DIM // 128, DIM // CH, 128, CH],
                               bf16, kind="ExternalInput").ap()
    bproj_ext = nc.dram_tensor("bproj", [1, DIM], f32,
                               kind="ExternalInput").ap()
    out_ext = nc.dram_tensor("out", [RPC, DIM], bf16,
                             kind="ExternalOutput").ap()

    NDT = DIM // 128          # 16 contraction tiles
    NSC = S // CH             # 4 s-chunks per batch
    NQKM = 2 * DIM // 128     # 32 q/k output M-tiles (dest-grouped)
    rg = [list(range(NCORES))]

    with tile.TileContext(nc) as tc, ExitStack() as top:
        dram = top.enter_context(tc.tile_pool(name="dram", bufs=1,
                                              space="DRAM"))
        q_send = dram.tile([DIM, RPC], bf16, name="q_send")
        k_sendA = dram.tile([NCORES * HD, RPC], bf16, name="k_sendA")
        k_sendB = dram.tile([NCORES * HD, RPC], bf16, name="k_sendB")
        v_send = dram.tile([NCORES, 128, HPC, RPC // 128, HD], bf16,
                           name="v_send")
        q_out = dram.tile([DIM, RPC], bf16, name="q_out")
        k_outA = dram.tile([NCORES * HD, RPC], bf16, name="k_outA")
        k_outB = dram.tile([NCORES * HD, RPC], bf16, name="k_outB")
        v_out = dram.tile([NCORES, 128, HPC, RPC // 128, HD], bf16,
                          name="v_out")
        a2a_in = [dram.tile([DIM, RPC // 2], bf16, name=f"a2a_in{i}")
                  for i in range(2)]
        a2a_out = [dram.tile([DIM, RPC // 2], bf16, name=f"a2a_out{i}")
                   for i in range(2)]

        const = top.enter_context(tc.tile_pool(name="const", bufs=1))
        ones = const.tile([128, 128], bf16, name="ones", tag="ones")
        nc.any.memset(ones[:], 1.0)
        msk = []
        for i in range(n_mask_tiles):
            w = mask_widths[i]
            mt = const.tile([128, w], bf16, name=f"msk{i}", tag=f"msk{i}")
            msk.append(mt)
        # broadcast biases (filled early in phase 1; bf16 is plenty
        # for a +bias epilogue)
        bvb = const.tile([128, DIM], bf16, name="bvb", tag="bvb")
        bpb = const.tile([128, DIM], bf16, name="bpb", tag="bpb")

        # persistent qkv storage for the attention phase (bf16); loads are
        # issued during phase 1 as soon as each AllToAll lands.
        qT = [[None] * HPC for _ in range(B)]
        kT = [[None] * HPC for _ in range(B)]
        vS = [[None] * HPC for _ in range(B)]
        frees = []

        def alloc_bh(b, h):
            t1, f1 = tc.tile([128, S], bf16, name=f"qT{b}{h}")
            t2, f2 = tc.tile([128, S], bf16, name=f"kT{b}{h}")
            t3, f3 = tc.tile([128, S], bf16, name=f"vS{b}{h}")
            qT[b][h], kT[b][h], vS[b][h] = t1, t2, t3
            frees.extend([f1, f2, f3])

        for b in range(B):
            alloc_bh(b, 0)

        # ================= Phase 1: data-parallel QKV projection ======
        with ExitStack() as p1:
            biasp = p1.enter_context(tc.tile_pool(name="biasp", bufs=1))
            bp1 = biasp.tile([1, DIM], f32, name="bp1", tag="bp1")
            nc.sync.dma_start(out=bp1[:], in_=bproj_ext[:, :])
            bv1 = biasp.tile([1, DIM], f32, name="bv1", tag="bv1")
            nc.sync.dma_start(out=bv1[:], in_=bv_ext[:, :])
            bv1h = biasp.tile([1, DIM], bf16, name="bv1h", tag="bv1h")
            nc.vector.tensor_copy(bv1h[:], bv1[:])
            nc.gpsimd.partition_broadcast(bvb[:], bv1h[:])
            bp1h = biasp.tile([1, DIM], bf16, name="bp1h", tag="bp1h")
            nc.vector.tensor_copy(bp1h[:], bp1[:])
            nc.gpsimd.partition_broadcast(bpb[:], bp1h[:])
            xtp = p1.enter_context(tc.tile_pool(name="xtp", bufs=1))
            xT = xtp.tile([128, NDT, RPC], bf16, name="xT", tag="xT")
            nc.sync.dma_start(out=xT[:, 0:NDT // 2, :],
                              in_=xt_ext[:, 0:NDT // 2, :])
            nc.gpsimd.dma_start(out=xT[:, NDT // 2:NDT, :],
                               in_=xt_ext[:, NDT // 2:NDT, :])
            for i in range(n_mask_tiles):
                nc.sync.dma_start(out=msk[i][:],
                                  in_=maskt_ext[i, :, :mask_widths[i]])

            psA = p1.enter_context(tc.tile_pool(name="psA", bufs=3,
                                                space="PSUM"))
            psV = p1.enter_context(tc.tile_pool(name="psV", bufs=3,
                                                space="PSUM"))
            bqkp = p1.enter_context(tc.tile_pool(name="bqkp", bufs=1))
            bmall = bqkp.tile([128, NQKM], f32, name="bmall", tag="bmall")
            nc.sync.dma_start(out=bmall[:], in_=bqk_ext[:, :])
            wqkp = p1.enter_context(tc.tile_pool(name="wqkp", bufs=8))
            qksb = p1.enter_context(tc.tile_pool(name="qksb", bufs=12))
            wvp = p1.enter_context(tc.tile_pool(name="wvp", bufs=1))
            vsb = p1.enter_context(tc.tile_pool(name="vsb", bufs=8))

            # --- v first: its AllToAll must land before attention starts
            for dhc in range(DIM // CH):
                wvs = []
                for dt_i in range(NDT):
                    wv = wvp.tile([128, CH], bf16, name=f"wv{dt_i}",
                                  tag=f"wv{dt_i}")
                    nc.sync.dma_start(out=wv[:], in_=wv_ext[dhc, dt_i])
                    wvs.append(wv)
                for tt in range(RPC // 128):
                    ps = psV.tile([128, CH], f32, name="psv", tag="V")
                    for dt_i in range(NDT):
                        nc.tensor.matmul(
                            ps[:],
                            xT[:, dt_i, tt * 128:(tt + 1) * 128],
                            wvs[dt_i][:],
                            start=(dt_i == 0), stop=(dt_i == NDT - 1))
                    sb = vsb.tile([128, CH], bf16, name="vsbt", tag="vsbt")
                    nc.vector.scalar_tensor_tensor(
                        out=sb[:], in0=ps[:], scalar=1.0,
                        in1=bvb[:, dhc * CH:(dhc + 1) * CH],
                        op0=ALU.mult, op1=ALU.add)
                    # split the 512 v-dims into the two destination
                    # blocks, stored so the attention-side read of each
                    # head is a fully contiguous [128, 1024] block
                    for half in range(2):
                        dest = 2 * dhc + half
                        for hp in range(HPC):
                            nc.scalar.dma_start(
                                out=v_send[dest, :, hp, tt, :],
                                in_=sb[:, (2 * half + hp) * HD:
                                       (2 * half + hp + 1) * HD])
            nc.gpsimd.collective_compute(
                "AllToAll", mybir.AluOpType.bypass, replica_groups=rg,
                ins=[v_send.opt()], outs=[v_out.opt()])

            def qk_tile(m):
                # one dest-grouped M-tile of [dim, 128]; m<16: q,
                # 16..23: k head-0 half, 24..31: k head-1 half
                wm = wqkp.tile([128, DIM], bf16, name="wm", tag="wm")
                nc.sync.dma_start(
                    out=wm[:].rearrange("p (dt c) -> p dt c", dt=NDT),
                    in_=wqk_ext[m])
                if m < NQKM // 2:
                    dst, mm = q_send, m
                elif m < 3 * NQKM // 4:
                    dst, mm = k_sendA, m - NQKM // 2
                else:
                    dst, mm = k_sendB, m - 3 * NQKM // 4
                for nchk in range(RPC // CH):
                    ps = psA.tile([128, CH], f32, name="psqk", tag="A")
                    for dt_i in range(NDT):
                        nc.tensor.matmul(
                            ps[:], wm[:, dt_i * 128:(dt_i + 1) * 128],
                            xT[:, dt_i, nchk * CH:(nchk + 1) * CH],
                            start=(dt_i == 0), stop=(dt_i == NDT - 1))
                    sb = qksb.tile([128, CH], bf16, name="sb", tag="sb")
                    nc.vector.tensor_scalar_add(sb[:], ps[:], bmall[:, m:m + 1])
                    nc.scalar.dma_start(
                        out=dst[mm * 128:(mm + 1) * 128,
                                nchk * CH:(nchk + 1) * CH],
                        in_=sb[:])

            for m in range(NQKM // 2):
                qk_tile(m)
            nc.gpsimd.collective_compute(
                "AllToAll", mybir.AluOpType.bypass, replica_groups=rg,
                ins=[q_send.opt()], outs=[q_out.opt()])
            for b in range(B):
                for sh in range(2):
                    nc.gpsimd.dma_start(
                        out=vS[b][0][:, sh * RPC:(sh + 1) * RPC],
                        in_=v_out[2 * b + sh, :, 0, :, :])
            for m in range(NQKM // 2, 3 * NQKM // 4):
                qk_tile(m)
            nc.gpsimd.collective_compute(
                "AllToAll", mybir.AluOpType.bypass, replica_groups=rg,
                ins=[k_sendA.opt()], outs=[k_outA.opt()])
            for b in range(B):
                for sh in range(2):
                    r0 = (2 * b + sh) * (HPC * HD)
                    nc.gpsimd.dma_start(
                        out=qT[b][0][:, sh * RPC:(sh + 1) * RPC],
                        in_=q_out[r0:r0 + HD, :])
            for b in range(B):
                for sh in range(2):
                    src = 2 * b + sh
                    nc.gpsimd.dma_start(
                        out=kT[b][0][:, sh * RPC:(sh + 1) * RPC],
                        in_=k_outA[src * HD:(src + 1) * HD, :])
            for m in range(3 * NQKM // 4, NQKM):
                qk_tile(m)
            nc.gpsimd.collective_compute(
                "AllToAll", mybir.AluOpType.bypass, replica_groups=rg,
                ins=[k_sendB.opt()], outs=[k_outB.opt()])


        # head-1 attention tiles are allocated only now, after phase-1
        # staging pools have closed (frees 48KB/partition during phase 1)
        for b in range(B):
            alloc_bh(b, 1)
        for b in range(B):
            for sh in range(2):
                nc.gpsimd.dma_start(
                    out=vS[b][1][:, sh * RPC:(sh + 1) * RPC],
                    in_=v_out[2 * b + sh, :, 1, :, :])
                r0 = (2 * b + sh) * (HPC * HD)
                nc.gpsimd.dma_start(
                    out=qT[b][1][:, sh * RPC:(sh + 1) * RPC],
                    in_=q_out[r0 + HD:r0 + 2 * HD, :])
        for b in range(B):
            for sh in range(2):
                src = 2 * b + sh
                nc.gpsimd.dma_start(
                    out=kT[b][1][:, sh * RPC:(sh + 1) * RPC],
                    in_=k_outB[src * HD:(src + 1) * HD, :])

        # ================= Phase 2: attention + interleaved proj ======
        with ExitStack() as p2:
            psG = p2.enter_context(tc.tile_pool(name="psG", bufs=2,
                                                space="PSUM"))
            psO = p2.enter_context(tc.tile_pool(name="psO", bufs=2,
                                                space="PSUM"))
            psS = p2.enter_context(tc.tile_pool(name="psS", bufs=2,
                                                space="PSUM"))
            ptp = p2.enter_context(tc.tile_pool(name="ptp", bufs=5))
            recp = p2.enter_context(tc.tile_pool(name="recp", bufs=2))
            otp = p2.enter_context(tc.tile_pool(name="otp", bufs=5))
            # output projection weights, preloaded during attention
            wpp = p2.enter_context(tc.tile_pool(name="wpp", bufs=1))
            wps = []  # [ot][nchunk] -> [128, CH]
            for ot in range(NDT):
                row = []
                for nchunk in range(DIM // CH):
                    wp = wpp.tile([128, CH], bf16, name=f"wp{ot}_{nchunk}",
                                  tag=f"wp{ot}_{nchunk}")
                    nc.sync.dma_start(out=wp[:],
                                      in_=wproj_ext[ot, nchunk])
                    row.append(wp)
                wps.append(row)
            ocp = p2.enter_context(tc.tile_pool(name="ocp", bufs=1))
            resp = p2.enter_context(tc.tile_pool(name="resp", bufs=3))

            state = {}  # (b,h,qc) -> (o_ps, s_sum)

            def front(b, h, qc, p0):
                ents = sched[qc]
                if p0 == 0:
                    state[(b, h, qc)] = (
                        psO.tile([128, CH], f32, name="o_ps", tag="O"),
                        psS.tile([128, CH], f32, name="s_sum", tag="Ssum"))
                n = min(2, len(ents) - p0)
                spsG = psG.tile([128, 2 * CH], f32, name="spsG", tag="G")
                pT = ptp.tile([128, 2 * CH], bf16, name="pT", tag="pT")
                g_lo = None
                for i in range(n):
                    (kc, j, q_lo, mid, c_lo, c_hi) = ents[p0 + i]
                    kv0 = kc * CH + j * SUB
                    off = i * CH
                    nc.tensor.matmul(
                        spsG[:, off + q_lo:off + CH],
                        kT[b][h][:, kv0:kv0 + SUB],
                        qT[b][h][:, qc * CH + q_lo:(qc + 1) * CH],
                        start=True, stop=True)
                    if mid >= 0:
                        nc.vector.tensor_add(
                            spsG[:, off + c_lo:off + c_hi],
                            spsG[:, off + c_lo:off + c_hi],
                            msk[mid][:, :c_hi - c_lo])
                    lo = off + q_lo
                    g_lo = lo if g_lo is None else min(g_lo, lo)
                nc.scalar.activation(
                    pT[:, g_lo:n * CH], spsG[:, g_lo:n * CH],
                    AF.Exp, scale=SCALE)
                return pT

            def back(b, h, qc, p0, pT):
                ents = sched[qc]
                last = len(ents) - 1
                o_ps, s_sum = state[(b, h, qc)]
                n = min(2, len(ents) - p0)
                for i in range(n):
                    ei = p0 + i
                    (kc, j, q_lo, mid, c_lo, c_hi) = ents[ei]
                    off = i * CH
                    nc.tensor.matmul(
                        s_sum[:, q_lo:CH], ones[:],
                        pT[:, off + q_lo:off + CH],
                        start=(ei == 0), stop=(ei == last))
                    kvt = kc * (CH // SUB) + j
                    nc.tensor.matmul(
                        o_ps[:, q_lo:CH],
                        vS[b][h][:, kvt * HD:(kvt + 1) * HD],
                        pT[:, off + q_lo:off + CH],
                        start=(ei == 0), stop=(ei == last))
                if p0 + n > last:  # query chunk complete
                    del state[(b, h, qc)]
                    rec = recp.tile([128, CH], f32, name="rec", tag="rec")
                    nc.vector.reciprocal_approx_fast(rec[:], s_sum[:])
                    oT = otp.tile([128, CH], bf16, name="oT", tag="oT")
                    nc.vector.tensor_mul(oT[:], o_ps[:], rec[:])
                    dest = 2 * b + qc // 2
                    r0 = dest * (HPC * HD) + h * HD
                    nc.scalar.dma_start(
                        out=a2a_in[qc % 2][r0:r0 + HD, :], in_=oT[:])

            pipe = []

            def run_units(items):
                # extend the single software pipeline (never drains
                # between sections)
                for (b, h, qcs) in items:
                    for qc in qcs:
                        for p0 in range(0, len(sched[qc]), 2):
                            u = (b, h, qc, p0)
                            pipe.append((u, front(*u)))
                            if len(pipe) > 1:
                                (ub, pT) = pipe.pop(0)
                                back(*ub, pT)

            def drain():
                while pipe:
                    (ub, pT) = pipe.pop(0)
                    back(*ub, pT)

            oc_t = [None] * NDT

            def proj_load(half):
                for ot in range(NDT):
                    t = ocp.tile([128, RPC // 2], bf16,
                                 name=f"oc{ot}", tag=f"oc{ot}")
                    nc.sync.dma_start(
                        out=t[:],
                        in_=a2a_out[half][ot * 128:(ot + 1) * 128, :])
                    oc_t[ot] = t

            def proj_rts(half, rts):
                # output projection for row-tiles rts of the given half;
                # PSUM comes from the psG ring (2x 512-wide chunks per
                # [128,1024] tile, bank-aligned).
                for rt in rts:
                    for pair in range(2):
                        t = psG.tile([128, 2 * CH], f32, name="spsG",
                                     tag="G")
                        for sub_i in range(2):
                            nchunk = pair * 2 + sub_i
                            for ot in range(NDT):
                                nc.tensor.matmul(
                                    t[:, sub_i * CH:(sub_i + 1) * CH],
                                    oc_t[ot][:, rt * 128:(rt + 1) * 128],
                                    wps[ot][nchunk][:],
                                    start=(ot == 0), stop=(ot == NDT - 1))
                        grow = half * (RPC // 2) + rt * 128
                        for sub_i in range(2):
                            nchunk = pair * 2 + sub_i
                            res = resp.tile([128, CH], bf16, name="res",
                                            tag="res")
                            nc.vector.scalar_tensor_tensor(
                                out=res[:],
                                in0=t[:, sub_i * CH:(sub_i + 1) * CH],
                                scalar=1.0,
                                in1=bpb[:, nchunk * CH:(nchunk + 1) * CH],
                                op0=ALU.mult, op1=ALU.add)
                            nc.scalar.dma_start(
                                out=out_ext[grow:grow + 128,
                                            nchunk * CH:(nchunk + 1) * CH],
                                in_=res[:])

            # S1: head 0, even query chunks; S2: head 1, even chunks
            run_units([(b, 0, (0, 2)) for b in range(B)])
            run_units([(b, 1, (0, 2)) for b in range(B)])
            drain()
            nc.gpsimd.collective_compute(
                "AllToAll", mybir.AluOpType.bypass, replica_groups=rg,
                ins=[a2a_in[0].opt()], outs=[a2a_out[0].opt()])
            # S3: head 0, odd chunks; then most of proj half-0 (its A2A
            # lands while S3 computes); S4: head 1, odd chunks.
            run_units([(b, 0, (1, 3)) for b in range(B)])
            run_units([(b, 1, (1, 3)) for b in range(B)])
            drain()
            nc.gpsimd.collective_compute(
                "AllToAll", mybir.AluOpType.bypass, replica_groups=rg,
                ins=[a2a_in[1].opt()], outs=[a2a_out[1].opt()])
            # proj half-0 (inputs long since landed) covers the final
            # A2A's transfer; half-1 follows with everything resident.
            proj_load(0)
            proj_rts(0, [0, 1, 2, 3])
            proj_load(1)
            proj_rts(1, [0, 1, 2, 3])

        for f in reversed(frees):
            f()

    nc.compile()
    return nc


def _get_program(sched, n_real, mask_widths):
    key = (str(sched), tuple(mask_widths))
    if key not in _prog_cache:
        _prog_cache[key] = _build_program(sched, n_real, mask_widths)
    return _prog_cache[key]


def kernel(x=None, mask=None, Wqkv=None, bqkv=None, Wproj=None, bproj=None,
           start_pos=0, **_unused):
    from concourse.bass_utils import run_bass_kernel_spmd

    x = np.ascontiguousarray(np.asarray(x, dtype=np.float32).reshape(TOK, DIM))
    mask = np.asarray(mask, dtype=np.float32)
    Wqkv = np.asarray(Wqkv, dtype=np.float32)
    bqkv = np.asarray(bqkv, dtype=np.float32)
    Wproj = np.asarray(Wproj, dtype=np.float32)
    bproj = np.asarray(bproj, dtype=np.float32)

    sched, mask_pack, widths, n_real = _analyze_mask(mask)
    nc = _get_program(sched, n_real, widths)

    # q/k weight columns reordered dest-major: for each destination core
    # d: [q_{2d}, q_{2d+1}, k_{2d}, k_{2d+1}]
    qk_cols = []
    for d in range(NCORES):
        for hh in (HPC * d, HPC * d + 1):
            qk_cols.append((hh * HD, (hh + 1) * HD))          # q
    for hl in range(HPC):
        for d in range(NCORES):
            hh = HPC * d + hl
            qk_cols.append((DIM + hh * HD, DIM + (hh + 1) * HD))  # k
    wqk = np.concatenate([Wqkv[:, a:b] for a, b in qk_cols], axis=1)
    bqk = np.concatenate([bqkv[a:b] for a, b in qk_cols])
    wv = Wqkv[:, 2 * DIM:]
    bv = bqkv[2 * DIM:]

    # pre-tile every weight on the host so each on-chip DMA is one
    # contiguous block (strided loads flood the DMA engines with
    # sub-512B descriptors and starve everything else)
    wqk_t = wqk.astype(_BF16).reshape(DIM // 128, 128, 2 * DIM // 128, 128)
    wqk_t = wqk_t.transpose(2, 1, 0, 3)            # [m, p, dt, c]
    wv_t = wv.astype(_BF16).reshape(DIM // 128, 128, DIM // CH, CH)
    wv_t = wv_t.transpose(2, 0, 1, 3)              # [dhc, dt, p, c]
    wp_t = Wproj.astype(_BF16).reshape(DIM // 128, 128, DIM // CH, CH)
    wp_t = wp_t.transpose(0, 2, 1, 3)              # [ot, nc, p, c]
    shared = {
        "wqk": np.ascontiguousarray(wqk_t),
        "wv": np.ascontiguousarray(wv_t),
        "bqk": np.ascontiguousarray(
            bqk.astype(np.float32).reshape(-1, 128).T),
        "bv": np.ascontiguousarray(bv.reshape(1, DIM)),
        "maskt": mask_pack,
        "wproj": np.ascontiguousarray(wp_t),
        "bproj": np.ascontiguousarray(bproj.reshape(1, DIM)),
    }
    in_maps = []
    for c in range(NCORES):
        m = dict(shared)
        xc = x[c * RPC:(c + 1) * RPC].T.astype(_BF16)
        m["xt"] = np.ascontiguousarray(
            xc.reshape(DIM // 128, 128, RPC).transpose(1, 0, 2))
        in_maps.append(m)

    import os
    kw = {}
    if os.environ.get("KERNEL_TRACE"):
        kw["trace"] = True
    res = run_bass_kernel_spmd(nc, in_maps, core_ids=list(range(NCORES)), **kw)
    globals()["LAST_RUN"] = res
    if getattr(res, "exec_time_ns", None):
        print(f"HW exec time: {res.exec_time_ns} ns")
    outs = [res.results[c]["out"].astype(np.float32) for c in range(NCORES)]
    full = np.concatenate(outs, axis=0).reshape(B, S, DIM)
    return full


# revision 20
# speedup vs baseline: 1.1212x; 1.0526x over previous
"""Distributed Trainium2 kernel for a dense-transformer attention block.

Math (matches the reference):
    xqkv = x @ Wqkv + bqkv ; split into q,k,v heads
    scores = (q k^T) / sqrt(HD) + mask ; attn = softmax(scores)
    o = attn @ v ; out = o @ Wproj + bproj

Parallelization over 8 NeuronCores:
  - QKV projection is DATA-parallel: each core projects its own 1/8 of
    the tokens against the FULL Wqkv.  An AllToAll then regroups q/k/v
    so each core holds 2 heads over ALL tokens (head-parallel
    attention).  A second AllToAll redistributes the per-head outputs
    so each core holds ALL head-dims for its 1/8 of the rows and runs
    the output projection; the host concatenates row blocks.

Performance structure (v2):
  - x is pre-cast + pre-transposed on the HOST, so the kernel's first
    matmul only waits for a single 4MiB DMA (~15us) instead of an
    on-chip cast/round-trip-transpose (~57us).
  - Phase-1 order is v -> A2A(v) -> q -> A2A(q) -> kA -> A2A(kA) -> kB
    -> A2A(kB): every input the first attention unit needs has finished
    its collective before the QKV matmuls end, so TensorE never idles
    between phases.  v's stationaries are xT slices reused 4x.
  - Attention runs as ONE continuous 2-deep software pipeline across
    all (b, head, chunk) sections; collectives are issued mid-stream
    and tracked by Tile semaphores.
  - Wproj is preloaded during attention; the first output-projection
    rows are interleaved into the attention tail so the final A2A's
    latency hides behind real matmuls.  Projection PSUM comes from the
    same pool ring as the attention score tiles (two 512-wide chunks
    per [128,1024] tile; matmuls never cross a PSUM bank).
  - Softmax row-sums are ones-stationary matmuls (broadcast across
    partitions); normalization uses the fast DVE reciprocal (~18-bit,
    plenty for the 2e-2 gate).
"""

import hashlib
import numpy as np
import ml_dtypes

B, S, DIM, NH = 4, 2048, 2048, 16
HD = DIM // NH  # 128
NCORES = 8
HPC = NH // NCORES          # heads per core = 2
TOK = B * S                 # 8192 tokens
RPC = TOK // NCORES         # rows (tokens) per core = 1024
CH = 512                    # attention chunk (q and kv)
SUB = 128                   # kv subtile
SCALE = 1.0 / float(np.sqrt(HD))

_BF16 = ml_dtypes.bfloat16

_prog_cache = {}


def _analyze_mask(mask):
    """Build the attention schedule from the additive mask.

    sched[qc] = list of (kc, j, q_lo, mask_id, c_lo, c_hi); mask_id is
    -1 when no mask add is needed for the entry.  Mask tiles are already
    transposed to [kv, q] layout and pre-divided by SCALE.
    """
    m = np.asarray(mask, dtype=np.float32).reshape(S, S)
    NEG = -1e8
    sched = []
    tiles = []
    tile_key = {}
    for qc in range(S // CH):
        ents = []
        for kc in range(S // CH):
            blk = m[qc * CH:(qc + 1) * CH, kc * CH:(kc + 1) * CH]
            if np.all(blk <= NEG):
                continue
            for j in range(CH // SUB):
                sub = blk[:, j * SUB:(j + 1) * SUB]       # [CH q, SUB kv]
                if np.all(sub <= NEG):
                    continue
                vis = ~np.all(sub <= NEG, axis=1)
                q_lo = int(np.argmax(vis))
                q_lo = (q_lo // SUB) * SUB
                if not ents:
                    q_lo = 0  # first entry must initialize full PSUM width
                nzrow = np.any(sub[q_lo:, :] != 0.0, axis=1)
                if nzrow.any():
                    first = q_lo + int(np.argmax(nzrow))
                    last = q_lo + len(nzrow) - int(np.argmax(nzrow[::-1]))
                    c_lo = (first // SUB) * SUB
                    c_hi = min(CH, ((last + SUB - 1) // SUB) * SUB)
                    content = np.ascontiguousarray(
                        (sub[c_lo:c_hi, :].T / SCALE).astype(_BF16))
                    key = (c_hi - c_lo,
                           hashlib.md5(content.tobytes()).hexdigest())
                    if key not in tile_key:
                        tile_key[key] = len(tiles)
                        tiles.append(content)
                    ents.append((kc, j, q_lo, tile_key[key], c_lo, c_hi))
                else:
                    ents.append((kc, j, q_lo, -1, 0, 0))
        assert ents, "a full query chunk is masked out; softmax undefined"
        sched.append(ents)
    n_real = len(tiles)
    widths = [t.shape[1] for t in tiles]
    pack = np.zeros((max(1, n_real), SUB, CH), dtype=_BF16)
    for i, t in enumerate(tiles):
        pack[i, :, :t.shape[1]] = t
    return sched, pack, widths, n_real


def _build_program(sched, n_mask_tiles, mask_widths):
    import concourse.bass as bass
    import concourse.tile as tile
    from concourse import bacc, mybir
    from contextlib import ExitStack

    f32 = mybir.dt.float32
    bf16 = mybir.dt.bfloat16
    AF = mybir.ActivationFunctionType
    ALU = mybir.AluOpType

    nc = bacc.Bacc("TRN2", target_bir_lowering=False, debug=False,
                   num_devices=NCORES)

    xt_ext = nc.dram_tensor("xt", [128, DIM // 128, RPC], bf16,
                            kind="ExternalInput").ap()
    wqk_ext = nc.dram_tensor("wqk", [2 * DIM // 128, 128, DIM // 128, 128],
                             bf16, kind="ExternalInput").ap()
    wv_ext = nc.dram_tensor("wv", [DIM // CH, DIM // 128, 128, CH], bf16,
                            kind="ExternalInput").ap()
    bqk_ext = nc.dram_tensor("bqk", [128, 2 * DIM // 128], f32,
                             kind="ExternalInput").ap()
    bv_ext = nc.dram_tensor("bv", [1, DIM], f32, kind="ExternalInput").ap()
    maskt_ext = nc.dram_tensor("maskt", [max(1, n_mask_tiles), SUB, CH], bf16,
                               kind="ExternalInput").ap()
    wproj_ext = nc.dram_tensor("wproj", [DIM // 128, DIM // CH, 128, CH],
                               bf16, kind="ExternalInput").ap()
    bproj_ext = nc.dram_tensor("bproj", [1, DIM], f32,
                               kind="ExternalInput").ap()
    out_ext = nc.dram_tensor("out", [RPC, DIM], bf16,
                             kind="ExternalOutput").ap()

    NDT = DIM // 128          # 16 contraction tiles
    NSC = S // CH             # 4 s-chunks per batch
    NQKM = 2 * DIM // 128     # 32 q/k output M-tiles (dest-grouped)
    rg = [list(range(NCORES))]

    with tile.TileContext(nc) as tc, ExitStack() as top:
        dram = top.enter_context(tc.tile_pool(name="dram", bufs=1,
                                              space="DRAM"))
        q_send = dram.tile([DIM, RPC], bf16, name="q_send")
        k_sendA = dram.tile([NCORES * HD, RPC], bf16, name="k_sendA")
        k_sendB = dram.tile([NCORES * HD, RPC], bf16, name="k_sendB")
        v_send = dram.tile([NCORES, 128, HPC, RPC // 128, HD], bf16,
                           name="v_send")
        q_out = dram.tile([DIM, RPC], bf16, name="q_out")
        k_outA = dram.tile([NCORES * HD, RPC], bf16, name="k_outA")
        k_outB = dram.tile([NCORES * HD, RPC], bf16, name="k_outB")
        v_out = dram.tile([NCORES, 128, HPC, RPC // 128, HD], bf16,
                          name="v_out")
        a2a_in = [dram.tile([DIM, RPC // 2], bf16, name=f"a2a_in{i}")
                  for i in range(2)]
        a2a_out = [dram.tile([DIM, RPC // 2], bf16, name=f"a2a_out{i}")
                   for i in range(2)]

        const = top.enter_context(tc.tile_pool(name="const", bufs=1))
        ones = const.tile([128, 128], bf16, name="ones", tag="ones")
        nc.any.memset(ones[:], 1.0)
        msk = []
        for i in range(n_mask_tiles):
            w = mask_widths[i]
            mt = const.tile([128, w], bf16, name=f"msk{i}", tag=f"msk{i}")
            msk.append(mt)
        # broadcast biases (filled early in phase 1; bf16 is plenty
        # for a +bias epilogue)
        bvb = const.tile([128, DIM], bf16, name="bvb", tag="bvb")
        bpb = const.tile([128, DIM], bf16, name="bpb", tag="bpb")

        # persistent qkv storage for the attention phase (bf16); loads are
        # issued during phase 1 as soon as each AllToAll lands.
        qT = [[None] * HPC for _ in range(B)]
        kT = [[None] * HPC for _ in range(B)]
        vS = [[None] * HPC for _ in range(B)]
        frees = []

        def alloc_bh(b, h):
            t1, f1 = tc.tile([128, S], bf16, name=f"qT{b}{h}")
            t2, f2 = tc.tile([128, S], bf16, name=f"kT{b}{h}")
            t3, f3 = tc.tile([128, S], bf16, name=f"vS{b}{h}")
            qT[b][h], kT[b][h], vS[b][h] = t1, t2, t3
            frees.extend([f1, f2, f3])

        for b in range(B):
            alloc_bh(b, 0)

        # ================= Phase 1: data-parallel QKV projection ======
        with ExitStack() as p1:
            biasp = p1.enter_context(tc.tile_pool(name="biasp", bufs=1))
            bp1 = biasp.tile([1, DIM], f32, name="bp1", tag="bp1")
            nc.sync.dma_start(out=bp1[:], in_=bproj_ext[:, :])
            bv1 = biasp.tile([1, DIM], f32, name="bv1", tag="bv1")
            nc.sync.dma_start(out=bv1[:], in_=bv_ext[:, :])
            bv1h = biasp.tile([1, DIM], bf16, name="bv1h", tag="bv1h")
            nc.vector.tensor_copy(bv1h[:], bv1[:])
            nc.gpsimd.partition_broadcast(bvb[:], bv1h[:])
            bp1h = biasp.tile([1, DIM], bf16, name="bp1h", tag="bp1h")
            nc.vector.tensor_copy(bp1h[:], bp1[:])
            nc.gpsimd.partition_broadcast(bpb[:], bp1h[:])
            xtp = p1.enter_context(tc.tile_pool(name="xtp", bufs=1))
            xT = xtp.tile([128, NDT, RPC], bf16, name="xT", tag="xT")
            nc.sync.dma_start(out=xT[:, 0:NDT // 2, :],
                              in_=xt_ext[:, 0:NDT // 2, :])
            nc.gpsimd.dma_start(out=xT[:, NDT // 2:NDT, :],
                               in_=xt_ext[:, NDT // 2:NDT, :])
            for i in range(n_mask_tiles):
                nc.sync.dma_start(out=msk[i][:],
                                  in_=maskt_ext[i, :, :mask_widths[i]])

            psA = p1.enter_context(tc.tile_pool(name="psA", bufs=3,
                                                space="PSUM"))
            psV = p1.enter_context(tc.tile_pool(name="psV", bufs=3,
                                                space="PSUM"))
            bqkp = p1.enter_context(tc.tile_pool(name="bqkp", bufs=1))
            bmall = bqkp.tile([128, NQKM], f32, name="bmall", tag="bmall")
            nc.sync.dma_start(out=bmall[:], in_=bqk_ext[:, :])
            wqkp = p1.enter_context(tc.tile_pool(name="wqkp", bufs=8))
            qksb = p1.enter_context(tc.tile_pool(name="qksb", bufs=12))
            wvp = p1.enter_context(tc.tile_pool(name="wvp", bufs=1))
            vsb = p1.enter_context(tc.tile_pool(name="vsb", bufs=8))

            # --- v first: its AllToAll must land before attention starts
            for dhc in range(DIM // CH):
                wvs = []
                for dt_i in range(NDT):
                    wv = wvp.tile([128, CH], bf16, name=f"wv{dt_i}",
                                  tag=f"wv{dt_i}")
                    nc.sync.dma_start(out=wv[:], in_=wv_ext[dhc, dt_i])
                    wvs.append(wv)
                for tt in range(RPC // 128):
                    ps = psV.tile([128, CH], f32, name="psv", tag="V")
                    for dt_i in range(NDT):
                        nc.tensor.matmul(
                            ps[:],
                            xT[:, dt_i, tt * 128:(tt + 1) * 128],
                            wvs[dt_i][:],
                            start=(dt_i == 0), stop=(dt_i == NDT - 1))
                    sb = vsb.tile([128, CH], bf16, name="vsbt", tag="vsbt")
                    nc.vector.scalar_tensor_tensor(
                        out=sb[:], in0=ps[:], scalar=1.0,
                        in1=bvb[:, dhc * CH:(dhc + 1) * CH],
                        op0=ALU.mult, op1=ALU.add)
                    # split the 512 v-dims into the two destination
                    # blocks, stored so the attention-side read of each
                    # head is a fully contiguous [128, 1024] block
                    for half in range(2):
                        dest = 2 * dhc + half
                        for hp in range(HPC):
                            nc.scalar.dma_start(
                                out=v_send[dest, :, hp, tt, :],
                                in_=sb[:, (2 * half + hp) * HD:
                                       (2 * half + hp + 1) * HD])
            nc.gpsimd.collective_compute(
                "AllToAll", mybir.AluOpType.bypass, replica_groups=rg,
                ins=[v_send.opt()], outs=[v_out.opt()])

            def qk_tile(m):
                # one dest-grouped M-tile of [dim, 128]; m<16: q,
                # 16..23: k head-0 half, 24..31: k head-1 half
                wm = wqkp.tile([128, DIM], bf16, name="wm", tag="wm")
                nc.sync.dma_start(
                    out=wm[:].rearrange("p (dt c) -> p dt c", dt=NDT),
                    in_=wqk_ext[m])
                if m < NQKM // 2:
                    dst, mm = q_send, m
                elif m < 3 * NQKM // 4:
                    dst, mm = k_sendA, m - NQKM // 2
                else:
                    dst, mm = k_sendB, m - 3 * NQKM // 4
                for nchk in range(RPC // CH):
                    ps = psA.tile([128, CH], f32, name="psqk", tag="A")
                    for dt_i in range(NDT):
                        nc.tensor.matmul(
                            ps[:], wm[:, dt_i * 128:(dt_i + 1) * 128],
                            xT[:, dt_i, nchk * CH:(nchk + 1) * CH],
                            start=(dt_i == 0), stop=(dt_i == NDT - 1))
                    sb = qksb.tile([128, CH], bf16, name="sb", tag="sb")
                    nc.vector.tensor_scalar_add(sb[:], ps[:], bmall[:, m:m + 1])
                    nc.scalar.dma_start(
                        out=dst[mm * 128:(mm + 1) * 128,
                                nchk * CH:(nchk + 1) * CH],
                        in_=sb[:])

            for m in range(NQKM // 2):
                qk_tile(m)
            nc.gpsimd.collective_compute(
                "AllToAll", mybir.AluOpType.bypass, replica_groups=rg,
                ins=[q_send.opt()], outs=[q_out.opt()])
            for b in range(B):
                for sh in range(2):
                    nc.gpsimd.dma_start(
                        out=vS[b][0][:, sh * RPC:(sh + 1) * RPC],
                        in_=v_out[2 * b + sh, :, 0, :, :])
            for m in range(NQKM // 2, 3 * NQKM // 4):
                qk_tile(m)
            nc.gpsimd.collective_compute(
                "AllToAll", mybir.AluOpType.bypass, replica_groups=rg,
                ins=[k_sendA.opt()], outs=[k_outA.opt()])
            for b in range(B):
                for sh in range(2):
                    r0 = (2 * b + sh) * (HPC * HD)
                    nc.gpsimd.dma_start(
                        out=qT[b][0][:, sh * RPC:(sh + 1) * RPC],
                        in_=q_out[r0:r0 + HD, :])
            for b in range(B):
                for sh in range(2):
                    src = 2 * b + sh
                    nc.gpsimd.dma_start(
                        out=kT[b][0][:, sh * RPC:(sh + 1) * RPC],
                        in_=k_outA[src * HD:(src + 1) * HD, :])
            for m in range(3 * NQKM // 4, NQKM):
                qk_tile(m)
            nc.gpsimd.collective_compute(
                "AllToAll", mybir.AluOpType.bypass, replica_groups=rg,
                ins=[k_sendB.opt()], outs=[k_outB.opt()])


        # head-1 attention tiles are allocated only now, after phase-1
        # staging pools have closed (frees 48KB/partition during phase 1)
        for b in range(B):
            alloc_bh(b, 1)
        for b in range(B):
            for sh in range(2):
                nc.gpsimd.dma_start(
                    out=vS[b][1][:, sh * RPC:(sh + 1) * RPC],
                    in_=v_out[2 * b + sh, :, 1, :, :])
                r0 = (2 * b + sh) * (HPC * HD)
                nc.gpsimd.dma_start(
                    out=qT[b][1][:, sh * RPC:(sh + 1) * RPC],
                    in_=q_out[r0 + HD:r0 + 2 * HD, :])
        for b in range(B):
            for sh in range(2):
                src = 2 * b + sh
                nc.gpsimd.dma_start(
                    out=kT[b][1][:, sh * RPC:(sh + 1) * RPC],
                    in_=k_outB[src * HD:(src + 1) * HD, :])

        # ================= Phase 2: attention + interleaved proj ======
        with ExitStack() as p2:
            psG = p2.enter_context(tc.tile_pool(name="psG", bufs=2,
                                                space="PSUM"))
            psO = p2.enter_context(tc.tile_pool(name="psO", bufs=2,
                                                space="PSUM"))
            psS = p2.enter_context(tc.tile_pool(name="psS", bufs=2,
                                                space="PSUM"))
            ptp = p2.enter_context(tc.tile_pool(name="ptp", bufs=4))
            recp = p2.enter_context(tc.tile_pool(name="recp", bufs=2))
            otp = p2.enter_context(tc.tile_pool(name="otp", bufs=8))
            # output projection weights, preloaded during attention
            wpp = p2.enter_context(tc.tile_pool(name="wpp", bufs=1))
            wps = []  # [ot][nchunk] -> [128, CH]
            for ot in range(NDT):
                row = []
                for nchunk in range(DIM // CH):
                    wp = wpp.tile([128, CH], bf16, name=f"wp{ot}_{nchunk}",
                                  tag=f"wp{ot}_{nchunk}")
                    nc.sync.dma_start(out=wp[:],
                                      in_=wproj_ext[ot, nchunk])
                    row.append(wp)
                wps.append(row)
            ocp = p2.enter_context(tc.tile_pool(name="ocp", bufs=1))
            resp = p2.enter_context(tc.tile_pool(name="resp", bufs=2))

            state = {}  # (b,h,qc) -> (o_ps, s_sum)

            def front(b, h, qc, p0):
                ents = sched[qc]
                if p0 == 0:
                    state[(b, h, qc)] = (
                        psO.tile([128, CH], f32, name="o_ps", tag="O"),
                        psS.tile([128, CH], f32, name="s_sum", tag="Ssum"))
                n = min(2, len(ents) - p0)
                spsG = psG.tile([128, 2 * CH], f32, name="spsG", tag="G")
                pT = ptp.tile([128, 2 * CH], bf16, name="pT", tag="pT")
                g_lo = None
                for i in range(n):
                    (kc, j, q_lo, mid, c_lo, c_hi) = ents[p0 + i]
                    kv0 = kc * CH + j * SUB
                    off = i * CH
                    nc.tensor.matmul(
                        spsG[:, off + q_lo:off + CH],
                        kT[b][h][:, kv0:kv0 + SUB],
                        qT[b][h][:, qc * CH + q_lo:(qc + 1) * CH],
                        start=True, stop=True)
                    if mid >= 0:
                        nc.vector.tensor_add(
                            spsG[:, off + c_lo:off + c_hi],
                            spsG[:, off + c_lo:off + c_hi],
                            msk[mid][:, :c_hi - c_lo])
                    lo = off + q_lo
                    g_lo = lo if g_lo is None else min(g_lo, lo)
                nc.scalar.activation(
                    pT[:, g_lo:n * CH], spsG[:, g_lo:n * CH],
                    AF.Exp, scale=SCALE)
                return pT

            def back(b, h, qc, p0, pT):
                ents = sched[qc]
                last = len(ents) - 1
                o_ps, s_sum = state[(b, h, qc)]
                n = min(2, len(ents) - p0)
                for i in range(n):
                    ei = p0 + i
                    (kc, j, q_lo, mid, c_lo, c_hi) = ents[ei]
                    off = i * CH
                    nc.tensor.matmul(
                        s_sum[:, q_lo:CH], ones[:],
                        pT[:, off + q_lo:off + CH],
                        start=(ei == 0), stop=(ei == last))
                    kvt = kc * (CH // SUB) + j
                    nc.tensor.matmul(
                        o_ps[:, q_lo:CH],
                        vS[b][h][:, kvt * HD:(kvt + 1) * HD],
                        pT[:, off + q_lo:off + CH],
                        start=(ei == 0), stop=(ei == last))
                if p0 + n > last:  # query chunk complete
                    del state[(b, h, qc)]
                    rec = recp.tile([128, CH], f32, name="rec", tag="rec")
                    nc.vector.reciprocal_approx_fast(rec[:], s_sum[:])
                    oT = otp.tile([128, CH], bf16, name="oT", tag="oT")
                    nc.vector.tensor_mul(oT[:], o_ps[:], rec[:])
                    dest = 2 * b + qc // 2
                    r0 = dest * (HPC * HD) + h * HD
                    nc.sync.dma_start(
                        out=a2a_in[qc % 2][r0:r0 + HD, :], in_=oT[:])

            pipe = []

            def run_units(items):
                # extend the single software pipeline (never drains
                # between sections)
                for (b, h, qcs) in items:
                    for qc in qcs:
                        for p0 in range(0, len(sched[qc]), 2):
                            u = (b, h, qc, p0)
                            pipe.append((u, front(*u)))
                            if len(pipe) > 1:
                                (ub, pT) = pipe.pop(0)
                                back(*ub, pT)

            def drain():
                while pipe:
                    (ub, pT) = pipe.pop(0)
                    back(*ub, pT)

            oc_t = [None] * NDT

            def proj_load(half):
                for ot in range(NDT):
                    t = ocp.tile([128, RPC // 2], bf16,
                                 name=f"oc{ot}", tag=f"oc{ot}")
                    nc.sync.dma_start(
                        out=t[:],
                        in_=a2a_out[half][ot * 128:(ot + 1) * 128, :])
                    oc_t[ot] = t

            def proj_rts(half, rts):
                # output projection for row-tiles rts of the given half;
                # PSUM comes from the psG ring (2x 512-wide chunks per
                # [128,1024] tile, bank-aligned).
                for rt in rts:
                    for pair in range(2):
                        t = psG.tile([128, 2 * CH], f32, name="spsG",
                                     tag="G")
                        for sub_i in range(2):
                            nchunk = pair * 2 + sub_i
                            for ot in range(NDT):
                                nc.tensor.matmul(
                                    t[:, sub_i * CH:(sub_i + 1) * CH],
                                    oc_t[ot][:, rt * 128:(rt + 1) * 128],
                                    wps[ot][nchunk][:],
                                    start=(ot == 0), stop=(ot == NDT - 1))
                        grow = half * (RPC // 2) + rt * 128
                        for sub_i in range(2):
                            nchunk = pair * 2 + sub_i
                            res = resp.tile([128, CH], bf16, name="res",
                                            tag="res")
                            nc.vector.scalar_tensor_tensor(
                                out=res[:],
                                in0=t[:, sub_i * CH:(sub_i + 1) * CH],
                                scalar=1.0,
                                in1=bpb[:, nchunk * CH:(nchunk + 1) * CH],
                                op0=ALU.mult, op1=ALU.add)
                            nc.scalar.dma_start(
                                out=out_ext[grow:grow + 128,
                                            nchunk * CH:(nchunk + 1) * CH],
                                in_=res[:])

            # S1: head 0, even query chunks; S2: head 1, even chunks
            run_units([(b, 0, (0, 2)) for b in range(B)])
            run_units([(b, 1, (0, 2)) for b in range(B)])
            drain()
            nc.gpsimd.collective_compute(
                "AllToAll", mybir.AluOpType.bypass, replica_groups=rg,
                ins=[a2a_in[0].opt()], outs=[a2a_out[0].opt()])
            # S3: head 0, odd chunks; then most of proj half-0 (its A2A
            # lands while S3 computes); S4: head 1, odd chunks.
            run_units([(b, 0, (1, 3)) for b in range(B)])
            run_units([(b, 1, (1, 3)) for b in range(B)])
            drain()
            nc.gpsimd.collective_compute(
                "AllToAll", mybir.AluOpType.bypass, replica_groups=rg,
                ins=[a2a_in[1].opt()], outs=[a2a_out[1].opt()])
            # proj half-0 (inputs long since landed) covers the final
            # A2A's transfer; half-1 follows with everything resident.
            proj_load(0)
            proj_rts(0, [0, 1, 2, 3])
            proj_load(1)
            proj_rts(1, [0, 1, 2, 3])

        for f in reversed(frees):
            f()

    nc.compile()
    return nc


def _get_program(sched, n_real, mask_widths):
    key = (str(sched), tuple(mask_widths))
    if key not in _prog_cache:
        _prog_cache[key] = _build_program(sched, n_real, mask_widths)
    return _prog_cache[key]


def kernel(x=None, mask=None, Wqkv=None, bqkv=None, Wproj=None, bproj=None,
           start_pos=0, **_unused):
    from concourse.bass_utils import run_bass_kernel_spmd

    x = np.ascontiguousarray(np.asarray(x, dtype=np.float32).reshape(TOK, DIM))
    mask = np.asarray(mask, dtype=np.float32)
    Wqkv = np.asarray(Wqkv, dtype=np.float32)
    bqkv = np.asarray(bqkv, dtype=np.float32)
    Wproj = np.asarray(Wproj, dtype=np.float32)
    bproj = np.asarray(bproj, dtype=np.float32)

    sched, mask_pack, widths, n_real = _analyze_mask(mask)
    nc = _get_program(sched, n_real, widths)

    # q/k weight columns reordered dest-major: for each destination core
    # d: [q_{2d}, q_{2d+1}, k_{2d}, k_{2d+1}]
    qk_cols = []
    for d in range(NCORES):
        for hh in (HPC * d, HPC * d + 1):
            qk_cols.append((hh * HD, (hh + 1) * HD))          # q
    for hl in range(HPC):
        for d in range(NCORES):
            hh = HPC * d + hl
            qk_cols.append((DIM + hh * HD, DIM + (hh + 1) * HD))  # k
    wqk = np.concatenate([Wqkv[:, a:b] for a, b in qk_cols], axis=1)
    bqk = np.concatenate([bqkv[a:b] for a, b in qk_cols])
    wv = Wqkv[:, 2 * DIM:]
    bv = bqkv[2 * DIM:]

    # pre-tile every weight on the host so each on-chip DMA is one
    # contiguous block (strided loads flood the DMA engines with
    # sub-512B descriptors and starve everything else)
    wqk_t = wqk.astype(_BF16).reshape(DIM // 128, 128, 2 * DIM // 128, 128)
    wqk_t = wqk_t.transpose(2, 1, 0, 3)            # [m, p, dt, c]
    wv_t = wv.astype(_BF16).reshape(DIM // 128, 128, DIM // CH, CH)
    wv_t = wv_t.transpose(2, 0, 1, 3)              # [dhc, dt, p, c]
    wp_t = Wproj.astype(_BF16).reshape(DIM // 128, 128, DIM // CH, CH)
    wp_t = wp_t.transpose(0, 2, 1, 3)              # [ot, nc, p, c]
    shared = {
        "wqk": np.ascontiguousarray(wqk_t),
        "wv": np.ascontiguousarray(wv_t),
        "bqk": np.ascontiguousarray(
            bqk.astype(np.float32).reshape(-1, 128).T),
        "bv": np.ascontiguousarray(bv.reshape(1, DIM)),
        "maskt": mask_pack,
        "wproj": np.ascontiguousarray(wp_t),
        "bproj": np.ascontiguousarray(bproj.reshape(1, DIM)),
    }
    in_maps = []
    for c in range(NCORES):
        m = dict(shared)
        xc = x[c * RPC:(c + 1) * RPC].T.astype(_BF16)
        m["xt"] = np.ascontiguousarray(
            xc.reshape(DIM // 128, 128, RPC).transpose(1, 0, 2))
        in_maps.append(m)

    import os
    kw = {}
    if os.environ.get("KERNEL_TRACE"):
        kw["trace"] = True
    res = run_bass_kernel_spmd(nc, in_maps, core_ids=list(range(NCORES)), **kw)
    globals()["LAST_RUN"] = res
    if getattr(res, "exec_time_ns", None):
        print(f"HW exec time: {res.exec_time_ns} ns")
    outs = [res.results[c]["out"].astype(np.float32) for c in range(NCORES)]
    full = np.concatenate(outs, axis=0).reshape(B, S, DIM)
    return full
